# revision 1
# baseline (speedup 1.0000x reference)
"""BitFeedForward TRN2 kernel: 8-way data-parallel over tokens.

Math (value-equivalent to reference):
  bitlinear(x, w, b) = act_quant(rmsnorm(x)) @ weight_quant(w).T + b
  -> n = round(x * 127/max|x|)          (exact ints, bf16)
     t = clip(round(w/mean|w|), -1, 1)  (ternary, fp8e4 - exact)
     h = (n @ t.T) * alpha + b,  alpha = mean|w| * rsqrt(mean(x^2)+eps) * max|x| / 127
  out = bitlinear(gelu(bitlinear(x,w1,b1)), w2, b2)

Structure per core (x [2048,1024]):
  phase A (per 128-token tile): quantize x -> n1 -> n1T (dma transpose);
    mm1 (bf16 x fp8, exact ints); gelu+alpha1 on ACT from PSUM; row absmax;
    quantize -> n2 (round step 1 on ACT via FMA, step 2 on DVE); n2 -> DRAM.
  phase B (per tile): transpose-load n2T from DRAM; Sum(n2^2) via PE gram diag
    (recovers mean(g^2) for rmsnorm alpha2); mm2; scale on ACT; store out.
  Weight prep: two-pass stream (pass1 abs-mean via ACT accum, pass2 ternarize
    on GPSIMD + DVE round, bf16 transpose via DMA xbar, fp8 cast). w2 prep is
    interleaved into phase A iterations so its DMA sits behind x/mm1-critical
    traffic but completes before phase B.
"""

import sys

sys.path.insert(0, "/opt/trn_rl_repo")

from contextlib import ExitStack

import numpy as np

import concourse.bass as bass
import concourse.tile as tile
from concourse import bacc, bass_utils, mybir
from concourse.masks import make_identity

F32 = mybir.dt.float32
BF16 = mybir.dt.bfloat16
FP8 = mybir.dt.float8e4
AX = mybir.AxisListType
OP = mybir.AluOpType
AF = mybir.ActivationFunctionType

RND = 12582912.0  # 1.5*2^23: +RND,-RND rounds fp32 to nearest int (RNE)
EPS = 1e-5
NCORES = 8


def build_bitffn(T_core=2048, D=1024, F=4096, has_b1=False, has_b2=False,
                 FB=4, XCH=2, WR1=2, LAG_P=6, USE_TTR=False, REPEAT=1,
                 QSPLIT=True, PMAX=True):
    TT = T_core // 128     # token tiles
    KD = D // 128          # k-subtiles for mm1
    KF = F // 128          # k-subtiles for mm2
    NF1 = F // 512         # mm1 psum chunks
    ND2 = D // 512         # mm2 psum chunks
    NXC = (TT + XCH - 1) // XCH          # x chunks
    NW1C = F // (128 * WR1)              # w1 chunks [128, WR1, D]
    W2CW = 2048 if F % 2048 == 0 else F  # w2 chunk width
    NW2H = F // W2CW
    NW2C = (D // 128) * NW2H             # w2 chunks [128, W2CW]

    nc = bacc.Bacc(
        "TRN2", target_bir_lowering=False, debug=False, enable_asserts=True
    )
    x_d = nc.dram_tensor("x", [T_core, D], F32, kind="ExternalInput").ap()
    w1_d = nc.dram_tensor("w1", [F, D], F32, kind="ExternalInput").ap()
    b1_d = nc.dram_tensor("b1", [1, F], F32, kind="ExternalInput").ap()
    w2_d = nc.dram_tensor("w2", [D, F], F32, kind="ExternalInput").ap()
    b2_d = nc.dram_tensor("b2", [1, D], F32, kind="ExternalInput").ap()
    out_d = nc.dram_tensor("out", [T_core, D], F32, kind="ExternalOutput").ap()
    n2_d = nc.dram_tensor("n2scratch", [T_core, F], BF16, kind="Internal").ap()
    # REPEAT-dependent input so distinct builds never collide in the
    # neuron compile cache (the custom_call HLO is otherwise shape-identical)
    rep_d = (nc.dram_tensor("reptag", [1, REPEAT], F32, kind="ExternalInput").ap()
             if REPEAT > 1 else None)

    with tile.TileContext(nc) as tc, ExitStack() as ctx:
        singles = ctx.enter_context(tc.tile_pool(name="singles", bufs=1))
        xp = ctx.enter_context(tc.tile_pool(name="xp", bufs=2))
        n1p = ctx.enter_context(tc.tile_pool(name="n1p", bufs=2))
        n1tp = ctx.enter_context(tc.tile_pool(name="n1tp", bufs=2 * XCH + 1))
        gp = ctx.enter_context(tc.tile_pool(name="gp", bufs=2))
        n2p = ctx.enter_context(tc.tile_pool(name="n2p", bufs=1))
        n2tp = ctx.enter_context(tc.tile_pool(name="n2tp", bufs=3))
        outp = ctx.enter_context(tc.tile_pool(name="outp", bufs=2))
        scp = ctx.enter_context(tc.tile_pool(name="scp", bufs=3))
        wstage = ctx.enter_context(tc.tile_pool(name="wstage", bufs=2))
        ps1 = ctx.enter_context(tc.tile_pool(name="ps1", bufs=FB, space="PSUM"))
        ps2 = ctx.enter_context(tc.tile_pool(name="ps2", bufs=ND2, space="PSUM"))
        sink = ctx.enter_context(tc.tile_pool(name="sink", bufs=1, space="PSUM"))
        psm = ctx.enter_context(tc.tile_pool(name="psm", bufs=1, space="PSUM"))

        # ---- persistent tiles ----
        w1qT = singles.tile([128, KD, F], FP8)     # 32KB/part
        w2qT = singles.tile([128, KF, D], FP8)     # 32KB/part
        ones_r = singles.tile([1, 128], F32)
        nc.vector.memset(ones_r, 1.0)
        ones_c = singles.tile([128, 1], F32)
        nc.vector.memset(ones_c, 1.0)
        if rep_d is not None:
            rtag = singles.tile([1, REPEAT], F32)
            nc.sync.dma_start(rtag, rep_d)

        if has_b1 or has_b2:
            ident = singles.tile([128, 128], F32)
            make_identity(nc, ident)
        amax1_a = singles.tile([128, TT], F32)
        ssq1_a = singles.tile([128, TT], F32)
        c1_a = singles.tile([128, TT], F32)
        al1_a = singles.tile([128, TT], F32)
        sx_a = singles.tile([128, TT, D // 512], F32)
        gmax_a = singles.tile([128, TT, NF1], F32)
        ssqn2_a = singles.tile([128, TT], F32)
        amax2_a = singles.tile([128, TT], F32)
        c2_a = singles.tile([128, TT], F32)
        al2_a = singles.tile([128, TT], F32)
        w1part = singles.tile([128, NW1C], F32)
        w2part = singles.tile([128, NW2C], F32)


        def finalize_mean(part_tile, nelem, name):
            """abs-sum partials [128, C] -> s_b = 1/clip(mean,1e-5) bcast,
            k_b = clip(mean)/127 bcast (both [128,1])."""
            rowsum = scp.tile([128, 1], F32, tag="w_rowsum")
            nc.vector.tensor_reduce(rowsum, part_tile, axis=AX.X, op=OP.add)
            tot_ps = psm.tile([128, 128], F32, tag="psm")
            nc.tensor.matmul(tot_ps[:1, :1], lhsT=rowsum, rhs=ones_c,
                             start=True, stop=True)
            mw = scp.tile([1, 1], F32, tag="w_mw")
            nc.vector.tensor_scalar(mw, tot_ps[:1, :1], 1.0 / nelem, 1e-5,
                                    op0=OP.mult, op1=OP.max)
            s = scp.tile([1, 1], F32, tag="w_s")
            nc.vector.reciprocal(s, mw)
            k = scp.tile([1, 1], F32, tag="w_k")
            nc.vector.tensor_scalar_mul(k, mw, 1.0 / 127.0)
            s_b = singles.tile([128, 1], F32, tag=f"{name}_sb")
            k_b = singles.tile([128, 1], F32, tag=f"{name}_kb")
            for src, dst in ((s, s_b), (k, k_b)):
                bps = psm.tile([128, 128], F32, tag="psm")
                nc.tensor.matmul(bps[:, :1], lhsT=ones_r, rhs=src,
                                 start=True, stop=True)
                nc.scalar.copy(dst, bps[:, :1])
            return s_b, k_b

        def col_to_row(col, tag):
            rp = psm.tile([128, 128], F32, tag="psm")
            nc.tensor.matmul(rp[:1, :], lhsT=col, rhs=ident,
                             start=True, stop=True)
            row = scp.tile([1, 128], F32, tag=tag, bufs=1)
            nc.scalar.copy(row, rp[:1, :])
            return row

        # ---- x prep: chunks of XCH token tiles ----
        n1T_aps = {}

        def x_prep_chunk(mc):
            m0 = mc * XCH
            nt = min(XCH, TT - m0)
            x_t = xp.tile([128, XCH, D], F32, tag="x")
            nc.sync.dma_start(
                x_t[:, :nt, :],
                x_d[m0 * 128:(m0 + nt) * 128, :].rearrange(
                    "(t p) d -> p t d", p=128),
            )
            nc.vector.tensor_reduce(
                amax1_a[:, m0:m0 + nt], x_t[:, :nt, :], axis=AX.X, op=OP.max,
                apply_absolute_value=True,
            )
            for t in range(nt):
                for j in range(D // 512):
                    sk = sink.tile([128, 512], F32, tag="sink")
                    nc.scalar.activation(
                        sk, x_t[:, t, j * 512:(j + 1) * 512], AF.Square,
                        accum_out=sx_a[:, m0 + t, j:j + 1],
                    )
            nc.vector.tensor_reduce(
                ssq1_a[:, m0:m0 + nt], sx_a[:, m0:m0 + nt, :],
                axis=AX.X, op=OP.add)
            rec = scp.tile([128, XCH], F32, tag="rec1")
            nc.vector.reciprocal(rec[:, :nt], amax1_a[:, m0:m0 + nt])
            nc.vector.tensor_scalar_mul(c1_a[:, m0:m0 + nt], rec[:, :nt], 127.0)
            for t in range(nt):
                m = m0 + t
                # round step 1 on ACT (fma: x*c1 + RND), in place
                nc.scalar.activation(x_t[:, t, :], x_t[:, t, :], AF.Copy,
                                     bias=RND, scale=c1_a[:, m:m + 1])
                n1 = n1p.tile([128, D], BF16, tag="n1")
                nc.vector.tensor_scalar(n1, x_t[:, t, :], -RND, None,
                                        op0=OP.add)
                n1T = n1tp.tile([128, KD, 128], BF16, tag="n1T")
                nc.sync.dma_start_transpose(n1T, n1)
                n1T_aps[m] = n1T

        # ---- weight chunk pipelines ----
        def w_abs_chunk(src_ap, part_col):
            """DMA f32 chunk + abs-sum partial into part_col [128,1]."""
            shp = src_ap.shape
            wid = int(np.prod(shp[1:]))
            wf = wstage.tile([128, wid], F32, tag="wf",
                             bufs=2 if (has_b1 or has_b2) else 3)
            dst = (wf.rearrange("p (a b) -> p a b", a=shp[1])
                   if len(shp) == 3 else wf)
            nc.sync.dma_start(dst, src_ap)
            nc.vector.tensor_reduce(part_col, wf, axis=AX.X, op=OP.add,
                                    apply_absolute_value=True)

        def tern_chunk(src_ap, s_b, dst_slices, cast_eng=None, ts_eng=None):
            """Load f32 chunk [128, n*1024], ternarize (gpsimd clip + DVE
            round), per-1024 bf16 transpose + fp8 cast into dst_slices."""
            shp = src_ap.shape
            wid = int(np.prod(shp[1:]))
            wf = wstage.tile([128, wid], F32, tag="wf",
                             bufs=2 if (has_b1 or has_b2) else 3)
            dst = (wf.rearrange("p (a b) -> p a b", a=shp[1])
                   if len(shp) == 3 else wf)
            nc.sync.dma_start(dst, src_ap)
            te = ts_eng or nc.vector
            te.tensor_scalar(wf, wf, s_b, 1.0, op0=OP.mult, op1=OP.min)
            te.tensor_scalar(wf, wf, -1.0, RND, op0=OP.max, op1=OP.add)
            wq = wstage.tile([128, wid], BF16, tag="wq", bufs=2)
            nc.vector.tensor_scalar(wq, wf, -RND, None, op0=OP.add)
            for i, dst_ap in enumerate(dst_slices):
                wtr = wstage.tile([128, 8, 128], BF16, tag="wtr", bufs=3)
                nc.sync.dma_start_transpose(wtr, wq[:, i * 1024:(i + 1) * 1024])
                (cast_eng or nc.scalar.copy)(dst_ap, wtr)

        def w1_src(c):
            return w1_d[c * 128 * WR1:(c + 1) * 128 * WR1, :].rearrange(
                "(t p) d -> p t d", p=128)

        def w1_dsts(c):
            return [w1qT[:, :, (c * WR1 + t) * 128:(c * WR1 + t + 1) * 128]
                    for t in range(WR1)]

        def w2_src(c):
            r, hh = divmod(c, NW2H)
            return w2_d[r * 128:(r + 1) * 128, hh * W2CW:(hh + 1) * W2CW]

        def w2_dsts(c):
            r, hh = divmod(c, NW2H)
            nsub = W2CW // 1024
            return [w2qT[:, (hh * nsub + t) * 8:(hh * nsub + t + 1) * 8,
                         r * 128:(r + 1) * 128] for t in range(nsub)]

        # ---- per-token-tile alpha1 ----
        def alpha1_smalls(m, kb1):
            u = scp.tile([128, 1], F32, tag="al_u")
            nc.vector.tensor_scalar(u, ssq1_a[:, m:m + 1], 1.0 / D, EPS,
                                    op0=OP.mult, op1=OP.add)
            v = scp.tile([128, 1], F32, tag="al_v")
            nc.vector.reciprocal(v, u)
            p = scp.tile([128, 1], F32, tag="al_p")
            nc.vector.tensor_tensor(p, amax1_a[:, m:m + 1], kb1, OP.mult)
            q = scp.tile([128, 1], F32, tag="al_q")
            nc.vector.tensor_tensor(q, p, p, OP.mult)
            z = scp.tile([128, 1], F32, tag="al_z")
            nc.vector.tensor_tensor(z, v, q, OP.mult)
            nc.scalar.sqrt(al1_a[:, m:m + 1], z)

        def mm1_side(m, kb1):
            alpha1_smalls(m, kb1)
            if has_b1:
                ia1 = scp.tile([128, 1], F32, tag="ia1")
                nc.vector.reciprocal(ia1, al1_a[:, m:m + 1])
                ia1_row = col_to_row(ia1, "ia1r")
            g_t = gp.tile([128, F], F32, tag="g")
            for fb in range(NF1 // FB):
                p1s = [ps1.tile([128, 512], F32, tag="ps1", name=f"p1_{fb}_{i}")
                       for i in range(FB)]
                for k in range(KD):
                    for fi in range(FB):
                        f = fb * FB + fi
                        nc.tensor.matmul(
                            p1s[fi], lhsT=n1T_aps[m][:, k, :],
                            rhs=w1qT[:, k, f * 512:(f + 1) * 512],
                            start=(k == 0), stop=(k == KD - 1 and not has_b1),
                        )
                for fi in range(FB):
                    f = fb * FB + fi
                    p1 = p1s[fi]
                    if has_b1:
                        bt1 = scp.tile([1, 512], F32, tag="bias", bufs=1)
                        nc.sync.dma_start(bt1, b1_d[:, f * 512:(f + 1) * 512])
                        nc.tensor.matmul(p1, lhsT=ia1_row, rhs=bt1,
                                         start=False, stop=True)
                    fs = slice(f * 512, (f + 1) * 512)
                    use_pmax = PMAX and not has_b1
                    if use_pmax:
                        # raw-h max straight off PSUM, in parallel with gelu:
                        # max|g| = gelu(max h) (gelu monotone, max h >> 0.35)
                        nc.vector.tensor_reduce(
                            gmax_a[:, m, f:f + 1], p1, axis=AX.X, op=OP.max)
                    nc.scalar.activation(g_t[:, fs], p1, AF.Gelu,
                                         scale=al1_a[:, m:m + 1])
                    if not use_pmax:
                        nc.vector.tensor_reduce(
                            gmax_a[:, m, f:f + 1], g_t[:, fs], axis=AX.X,
                            op=OP.max, apply_absolute_value=True)
            if PMAX and not has_b1:
                pmax = scp.tile([128, 1], F32, tag="pmax")
                nc.vector.tensor_reduce(
                    pmax, gmax_a[:, m, :], axis=AX.X, op=OP.max)
                nc.scalar.activation(amax2_a[:, m:m + 1], pmax, AF.Gelu,
                                     scale=al1_a[:, m:m + 1])
            else:
                nc.vector.tensor_reduce(
                    amax2_a[:, m:m + 1], gmax_a[:, m, :], axis=AX.X, op=OP.max)
            rec2 = scp.tile([128, 1], F32, tag="rec2")
            nc.vector.reciprocal(rec2, amax2_a[:, m:m + 1])
            nc.vector.tensor_scalar_mul(c2_a[:, m:m + 1], rec2, 127.0)
            # round trick, optionally split across DVE and ACT halves
            FH = F // 2 if QSPLIT else 0
            n2 = n2p.tile([128, F], BF16, tag="n2")
            if FH:
                nc.vector.tensor_scalar(g_t[:, :FH], g_t[:, :FH],
                                        c2_a[:, m:m + 1], RND,
                                        op0=OP.mult, op1=OP.add)
                nc.vector.tensor_scalar(n2[:, :FH], g_t[:, :FH], -RND, None,
                                        op0=OP.add)
            nc.scalar.activation(g_t[:, FH:], g_t[:, FH:], AF.Copy, bias=RND,
                                 scale=c2_a[:, m:m + 1])
            nc.scalar.activation(n2[:, FH:], g_t[:, FH:], AF.Copy, bias=-RND)
            nc.sync.dma_start(n2_d[m * 128:(m + 1) * 128, :], n2)
            # sum(n2^2): one full-row mul into dead g_t storage + one reduce
            if USE_TTR:
                nc.vector.tensor_tensor_reduce(
                    out=g_t, in0=n2, in1=n2, scale=1.0,
                    scalar=0.0, op0=OP.mult, op1=OP.add,
                    accum_out=ssqn2_a[:, m:m + 1],
                )
            else:
                nc.vector.tensor_tensor(g_t, n2, n2, OP.mult)
                nc.vector.tensor_reduce(ssqn2_a[:, m:m + 1], g_t,
                                        axis=AX.X, op=OP.add)

        # ---- phase B ----
        n2T_aps = {}

        def n2t_load(j):
            n2T = n2tp.tile([128, KF, 128], BF16, tag="n2T")
            nc.sync.dma_start_transpose(n2T, n2_d[j * 128:(j + 1) * 128, :])
            n2T_aps[j] = n2T

        def alpha2_smalls(j, kb2):
            """alpha2 = kb2*amax2*rsqrt(mean(g^2)+eps);
            Sum(g^2) ~= Sum(n2^2)*(amax2/127)^2, Sum(n2^2) from PE gram diag."""
            sq = ssqn2_a[:, j:j + 1]
            aa = scp.tile([128, 1], F32, tag="aa")
            nc.vector.tensor_tensor(aa, amax2_a[:, j:j + 1],
                                    amax2_a[:, j:j + 1], OP.mult)
            u = scp.tile([128, 1], F32, tag="al_u")
            nc.vector.tensor_tensor(u, sq, aa, OP.mult)
            nc.vector.tensor_scalar(u, u, 1.0 / (16129.0 * F), EPS,
                                    op0=OP.mult, op1=OP.add)
            v = scp.tile([128, 1], F32, tag="al_v")
            nc.vector.reciprocal(v, u)
            p = scp.tile([128, 1], F32, tag="al_p")
            nc.vector.tensor_tensor(p, amax2_a[:, j:j + 1], kb2, OP.mult)
            q = scp.tile([128, 1], F32, tag="al_q")
            nc.vector.tensor_tensor(q, p, p, OP.mult)
            z = scp.tile([128, 1], F32, tag="al_z")
            nc.vector.tensor_tensor(z, v, q, OP.mult)
            nc.scalar.sqrt(al2_a[:, j:j + 1], z)

        def mm2_side(j):
            n2T = n2T_aps.pop(j)
            if has_b2:
                ia2 = scp.tile([128, 1], F32, tag="ia2")
                nc.vector.reciprocal(ia2, al2_a[:, j:j + 1])
                ia2_row = col_to_row(ia2, "ia2r")
            o_t = outp.tile([128, D], F32, tag="o")
            p2s = [ps2.tile([128, 512], F32, tag="ps2", name=f"p2_{i}")
                   for i in range(ND2)]
            for k2 in range(KF):
                for d in range(ND2):
                    nc.tensor.matmul(
                        p2s[d], lhsT=n2T[:, k2, :],
                        rhs=w2qT[:, k2, d * 512:(d + 1) * 512],
                        start=(k2 == 0), stop=(k2 == KF - 1 and not has_b2),
                    )
            for d in range(ND2):
                p2 = p2s[d]
                if has_b2:
                    bt2 = scp.tile([1, 512], F32, tag="bias", bufs=1)
                    nc.sync.dma_start(bt2, b2_d[:, d * 512:(d + 1) * 512])
                    nc.tensor.matmul(p2, lhsT=ia2_row, rhs=bt2,
                                     start=False, stop=True)
                ds_ = slice(d * 512, (d + 1) * 512)
                nc.scalar.activation(o_t[:, ds_], p2, AF.Copy,
                                     scale=al2_a[:, j:j + 1])
            nc.sync.dma_start(out_d[j * 128:(j + 1) * 128, :], o_t)

        # ================= emission =================
        rep_ctx = tc.For_i(0, REPEAT, 1) if REPEAT > 1 else None
        if rep_ctx is not None:
            ctx.enter_context(rep_ctx)
        x_prep_chunk(0)
        # w1 pass 1
        for c in range(NW1C):
            w_abs_chunk(w1_src(c), w1part[:, c:c + 1])
        s1b, kb1 = finalize_mean(w1part, F * D, "w1")
        if NXC > 1:
            x_prep_chunk(1)
        # w1 pass 2
        for c in range(NW1C):
            tern_chunk(w1_src(c), s1b, w1_dsts(c))

        # phase A with interleaved w2 prep, phase B interleaved at LAG
        LAG = min(LAG_P, TT)
        LD = min(2, LAG)  # n2T load lead (iterations before its mm2)
        W2SPREAD = max(LAG - 1, 1)
        w2_per = max(1, (2 * NW2C + W2SPREAD - 1) // W2SPREAD)
        s2b = kb2 = None
        w2p1_next = 0
        w2p2_next = 0
        for it in range(TT + LAG):
            if it < TT:
                mc = it // XCH + 2
                if it % XCH == 0 and mc < NXC:
                    x_prep_chunk(mc)
                for _ in range(w2_per):
                    if w2p1_next < NW2C:
                        c = w2p1_next
                        w_abs_chunk(w2_src(c), w2part[:, c:c + 1])
                        w2p1_next += 1
                        if w2p1_next == NW2C:
                            s2b, kb2 = finalize_mean(w2part, D * F, "w2")
                    elif w2p2_next < NW2C:
                        tern_chunk(w2_src(w2p2_next), s2b,
                                   w2_dsts(w2p2_next),
                                   cast_eng=nc.gpsimd.tensor_copy,
                                   ts_eng=(nc.gpsimd if w2p2_next % 2 else None))
                        w2p2_next += 1
                mm1_side(it, kb1)
            jl = it - LAG + LD
            if 0 <= jl < TT and jl <= it:
                n2t_load(jl)
            j = it - LAG
            if 0 <= j < TT:
                alpha2_smalls(j, kb2)
                mm2_side(j)

    nc.compile()
    return nc


_NC_CACHE = {}


def _get_nc(T_core, D, F, has_b1, has_b2):
    key = (T_core, D, F, has_b1, has_b2)
    if key not in _NC_CACHE:
        _NC_CACHE[key] = build_bitffn(T_core, D, F, has_b1, has_b2)
    return _NC_CACHE[key]


def kernel(x, w1, b1, w2, b2):
    B, S, D = x.shape
    Fdim = w1.shape[0]
    T = B * S
    T_core = T // NCORES
    has_b1 = bool(np.any(b1))
    has_b2 = bool(np.any(b2))

    nc = _get_nc(T_core, D, Fdim, has_b1, has_b2)

    xf = np.ascontiguousarray(x.reshape(T, D).astype(np.float32))
    w1c = np.ascontiguousarray(w1.astype(np.float32))
    w2c = np.ascontiguousarray(w2.astype(np.float32))
    b1c = np.ascontiguousarray(b1.reshape(1, Fdim).astype(np.float32))
    b2c = np.ascontiguousarray(b2.reshape(1, D).astype(np.float32))

    in_maps = [
        {
            "x": xf[i * T_core:(i + 1) * T_core],
            "w1": w1c,
            "b1": b1c,
            "w2": w2c,
            "b2": b2c,
        }
        for i in range(NCORES)
    ]
    res = bass_utils.run_bass_kernel_spmd(
        nc, in_maps=in_maps, core_ids=list(range(NCORES))
    )
    out = np.concatenate([res.results[i]["out"] for i in range(NCORES)], axis=0)
    return out.reshape(B, S, D).astype(np.float32)



# revision 12
# speedup vs baseline: 1.9140x; 1.9140x over previous
"""BitFeedForward TRN2 kernel: 8-way data-parallel over tokens.

Math (value-equivalent to reference):
  bitlinear(x, w, b) = act_quant(rmsnorm(x)) @ weight_quant(w).T + b
  -> n = round(x * 127/max|x|)          (exact ints, bf16)
     t = clip(round(w/mean|w|), -1, 1)  (ternary, fp8e4 - exact)
     h = (n @ t.T) * alpha + b,  alpha = mean|w| * rsqrt(mean(x^2)+eps) * max|x| / 127
  out = bitlinear(gelu(bitlinear(x,w1,b1)), w2, b2)

Structure per core (x [2048,1024]):
  phase A (per 128-token tile): quantize x -> n1 -> n1T (dma transpose);
    mm1 (bf16 x fp8, exact ints); gelu+alpha1 on ACT from PSUM; amax2 via one
    DVE reduce over g; quantize -> n2 (round step 1 on ACT via FMA, step 2 on
    DVE); n2 -> n2T via direct SBUF->SBUF dma transpose (no DRAM roundtrip);
    ssq via DVE tensor_tensor_reduce into a dead sink.
  phase B (LAG=2 tiles behind): mm2 from n2T; scale on ACT; store out.
  Weight prep: two-pass stream (pass1 abs-mean via DVE reduce, pass2
  ternarize, bf16 transpose via DMA xbar, fp8 cast). w2 prep interleaved
  into phase A iterations.
"""

import sys

sys.path.insert(0, "/opt/trn_rl_repo")

from contextlib import ExitStack

import numpy as np

import concourse.bass as bass
import concourse.tile as tile
from concourse import bacc, bass_utils, mybir
from concourse.masks import make_identity

F32 = mybir.dt.float32
BF16 = mybir.dt.bfloat16
FP8 = mybir.dt.float8e4
AX = mybir.AxisListType
OP = mybir.AluOpType
AF = mybir.ActivationFunctionType

RND = 12582912.0  # 1.5*2^23: +RND,-RND rounds fp32 to nearest int (RNE)
EPS = 1e-5
NCORES = 8


def build_bitffn(T_core=2048, D=1024, F=4096, has_b1=False, has_b2=False,
                 FB=4, XCH=2, WR1=2, LAG_P=4, REPEAT=1, QSPLIT=True):
    TT = T_core // 128     # token tiles
    KD = D // 128          # k-subtiles for mm1
    KF = F // 128          # k-subtiles for mm2
    NF1 = F // 512         # mm1 psum chunks
    ND2 = D // 512         # mm2 psum chunks
    NXC = (TT + XCH - 1) // XCH          # x chunks
    NW1C = F // (128 * WR1)              # w1 chunks [128, WR1, D]
    W2CW = 2048 if F % 2048 == 0 else F  # w2 chunk width
    NW2H = F // W2CW
    NW2C = (D // 128) * NW2H             # w2 chunks [128, W2CW]

    nc = bacc.Bacc(
        "TRN2", target_bir_lowering=False, debug=False, enable_asserts=True
    )
    x_d = nc.dram_tensor("x", [T_core, D], F32, kind="ExternalInput").ap()
    w1_d = nc.dram_tensor("w1", [F, D], F32, kind="ExternalInput").ap()
    b1_d = nc.dram_tensor("b1", [1, F], F32, kind="ExternalInput").ap()
    w2_d = nc.dram_tensor("w2", [D, F], F32, kind="ExternalInput").ap()
    b2_d = nc.dram_tensor("b2", [1, D], F32, kind="ExternalInput").ap()
    out_d = nc.dram_tensor("out", [T_core, D], F32, kind="ExternalOutput").ap()
    # REPEAT-dependent input so distinct builds never collide in the
    # neuron compile cache (the custom_call HLO is otherwise shape-identical)
    rep_d = (nc.dram_tensor("reptag", [1, REPEAT], F32, kind="ExternalInput").ap()
             if REPEAT > 1 else None)

    with tile.TileContext(nc) as tc, ExitStack() as ctx:
        singles = ctx.enter_context(tc.tile_pool(name="singles", bufs=1))
        xp = ctx.enter_context(tc.tile_pool(name="xp", bufs=2))
        n1p = ctx.enter_context(tc.tile_pool(name="n1p", bufs=2))
        n1tp = ctx.enter_context(tc.tile_pool(name="n1tp", bufs=2 * XCH + 1))
        gp = ctx.enter_context(tc.tile_pool(name="gp", bufs=1))
        n2p = ctx.enter_context(tc.tile_pool(name="n2p", bufs=1))
        n2tp = ctx.enter_context(tc.tile_pool(name="n2tp", bufs=LAG_P + 1))
        outp = ctx.enter_context(tc.tile_pool(name="outp", bufs=2))
        scp = ctx.enter_context(tc.tile_pool(name="scp", bufs=3))
        wstage = ctx.enter_context(tc.tile_pool(name="wstage", bufs=2))
        ps1 = ctx.enter_context(tc.tile_pool(name="ps1", bufs=FB, space="PSUM"))
        ps2 = ctx.enter_context(tc.tile_pool(name="ps2", bufs=ND2, space="PSUM"))
        psm = ctx.enter_context(tc.tile_pool(name="psm", bufs=1, space="PSUM"))

        # ---- persistent tiles ----
        w1qT = singles.tile([128, KD, F], FP8)     # 32KB/part
        w2qT = singles.tile([128, KF, D], FP8)     # 32KB/part
        ones_r = singles.tile([1, 128], F32)
        nc.vector.memset(ones_r, 1.0)
        ones_c = singles.tile([128, 1], F32)
        nc.vector.memset(ones_c, 1.0)
        if rep_d is not None:
            rtag = singles.tile([1, REPEAT], F32)
            nc.sync.dma_start(rtag, rep_d)

        if has_b1 or has_b2:
            ident = singles.tile([128, 128], F32)
            make_identity(nc, ident)
        amax1_a = singles.tile([128, TT], F32)
        ssq1_a = singles.tile([128, TT], F32)
        c1_a = singles.tile([128, TT], F32)
        al1_a = singles.tile([128, TT], F32)
        ssqn2_a = singles.tile([128, TT], F32)
        amax2_a = singles.tile([128, TT], F32)
        c2_a = singles.tile([128, TT], F32)
        al2_a = singles.tile([128, TT], F32)
        w1part = singles.tile([128, NW1C], F32)
        w2part = singles.tile([128, NW2C], F32)
        ssink = singles.tile([128, F], BF16)       # dead TTR out store

        def finalize_mean(part_tile, nelem, name):
            """abs-sum partials [128, C] -> s_b = 1/clip(mean,1e-5) bcast,
            k_b = clip(mean)/127 bcast (both [128,1])."""
            rowsum = scp.tile([128, 1], F32, tag="w_rowsum")
            nc.vector.tensor_reduce(rowsum, part_tile, axis=AX.X, op=OP.add)
            tot_ps = psm.tile([128, 128], F32, tag="psm")
            nc.tensor.matmul(tot_ps[:1, :1], lhsT=rowsum, rhs=ones_c,
                             start=True, stop=True)
            mw = scp.tile([1, 1], F32, tag="w_mw")
            nc.vector.tensor_scalar(mw, tot_ps[:1, :1], 1.0 / nelem, 1e-5,
                                    op0=OP.mult, op1=OP.max)
            s = scp.tile([1, 1], F32, tag="w_s")
            nc.vector.reciprocal(s, mw)
            k = scp.tile([1, 1], F32, tag="w_k")
            nc.vector.tensor_scalar_mul(k, mw, 1.0 / 127.0)
            s_b = singles.tile([128, 1], F32, tag=f"{name}_sb")
            k_b = singles.tile([128, 1], F32, tag=f"{name}_kb")
            for src, dst in ((s, s_b), (k, k_b)):
                bps = psm.tile([128, 128], F32, tag="psm")
                nc.tensor.matmul(bps[:, :1], lhsT=ones_r, rhs=src,
                                 start=True, stop=True)
                nc.scalar.copy(dst, bps[:, :1])
            return s_b, k_b

        def col_to_row(col, tag):
            rp = psm.tile([128, 128], F32, tag="psm")
            nc.tensor.matmul(rp[:1, :], lhsT=col, rhs=ident,
                             start=True, stop=True)
            row = scp.tile([1, 128], F32, tag=tag, bufs=1)
            nc.scalar.copy(row, rp[:1, :])
            return row

        # ---- x prep: chunks of XCH token tiles ----
        n1T_aps = {}

        def x_prep_chunk(mc):
            m0 = mc * XCH
            nt = min(XCH, TT - m0)
            x_t = xp.tile([128, XCH, D], F32, tag="x")
            nc.sync.dma_start(
                x_t[:, :nt, :],
                x_d[m0 * 128:(m0 + nt) * 128, :].rearrange(
                    "(t p) d -> p t d", p=128),
            )
            nc.vector.tensor_reduce(
                amax1_a[:, m0:m0 + nt], x_t[:, :nt, :], axis=AX.X, op=OP.max,
                apply_absolute_value=True,
            )
            rec = scp.tile([128, XCH], F32, tag="rec1")
            nc.vector.reciprocal(rec[:, :nt], amax1_a[:, m0:m0 + nt])
            nc.vector.tensor_scalar_mul(c1_a[:, m0:m0 + nt], rec[:, :nt], 127.0)
            for t in range(nt):
                m = m0 + t
                n1 = n1p.tile([128, D], BF16, tag="n1")
                # ssq1 = sum(x^2): square into n1 (bf16 scratch, overwritten
                # by round step 2 below; same-engine WAW keeps order), reduce
                nc.vector.tensor_tensor(n1, x_t[:, t, :], x_t[:, t, :],
                                        OP.mult)
                nc.vector.tensor_reduce(ssq1_a[:, m:m + 1], n1, axis=AX.X,
                                        op=OP.add)
                # round step 1 on ACT (fma: x*c1 + RND), in place
                nc.scalar.activation(x_t[:, t, :], x_t[:, t, :], AF.Copy,
                                     bias=RND, scale=c1_a[:, m:m + 1])
                nc.vector.tensor_scalar(n1, x_t[:, t, :], -RND, None,
                                        op0=OP.add)
                n1T = n1tp.tile([128, KD, 128], BF16, tag="n1T")
                nc.sync.dma_start_transpose(n1T, n1)
                n1T_aps[m] = n1T

        # ---- weight chunk pipelines ----
        def w_abs_chunk(src_ap, part_col):
            """DMA f32 chunk + abs-sum partial into part_col [128,1]."""
            shp = src_ap.shape
            wid = int(np.prod(shp[1:]))
            wf = wstage.tile([128, wid], F32, tag="wf", bufs=2)
            dst = (wf.rearrange("p (a b) -> p a b", a=shp[1])
                   if len(shp) == 3 else wf)
            nc.sync.dma_start(dst, src_ap)
            nc.vector.tensor_reduce(part_col, wf, axis=AX.X, op=OP.add,
                                    apply_absolute_value=True)

        def tern_chunk(src_ap, s_b, dst_slices, cast_eng=None, ts_eng=None):
            """Load f32 chunk [128, n*1024], ternarize (clip + DVE round),
            per-1024 bf16 transpose + fp8 cast into dst_slices."""
            shp = src_ap.shape
            wid = int(np.prod(shp[1:]))
            wf = wstage.tile([128, wid], F32, tag="wf", bufs=2)
            dst = (wf.rearrange("p (a b) -> p a b", a=shp[1])
                   if len(shp) == 3 else wf)
            nc.sync.dma_start(dst, src_ap)
            te = ts_eng or nc.vector
            te.tensor_scalar(wf, wf, s_b, 1.0, op0=OP.mult, op1=OP.min)
            te.tensor_scalar(wf, wf, -1.0, RND, op0=OP.max, op1=OP.add)
            wq = wstage.tile([128, wid], BF16, tag="wq", bufs=2)
            nc.vector.tensor_scalar(wq, wf, -RND, None, op0=OP.add)
            for i, dst_ap in enumerate(dst_slices):
                wtr = wstage.tile([128, 8, 128], BF16, tag="wtr", bufs=2)
                nc.sync.dma_start_transpose(wtr, wq[:, i * 1024:(i + 1) * 1024])
                (cast_eng or nc.scalar.copy)(dst_ap, wtr)

        def w1_src(c):
            return w1_d[c * 128 * WR1:(c + 1) * 128 * WR1, :].rearrange(
                "(t p) d -> p t d", p=128)

        def w1_dsts(c):
            return [w1qT[:, :, (c * WR1 + t) * 128:(c * WR1 + t + 1) * 128]
                    for t in range(WR1)]

        def w2_src(c):
            r, hh = divmod(c, NW2H)
            return w2_d[r * 128:(r + 1) * 128, hh * W2CW:(hh + 1) * W2CW]

        def w2_dsts(c):
            r, hh = divmod(c, NW2H)
            nsub = W2CW // 1024
            return [w2qT[:, (hh * nsub + t) * 8:(hh * nsub + t + 1) * 8,
                         r * 128:(r + 1) * 128] for t in range(nsub)]

        # ---- per-token-tile alpha1 ----
        def alpha1_smalls(m, kb1):
            u = scp.tile([128, 1], F32, tag="al_u")
            nc.vector.tensor_scalar(u, ssq1_a[:, m:m + 1], 1.0 / D, EPS,
                                    op0=OP.mult, op1=OP.add)
            v = scp.tile([128, 1], F32, tag="al_v")
            nc.vector.reciprocal(v, u)
            p = scp.tile([128, 1], F32, tag="al_p")
            nc.vector.tensor_tensor(p, amax1_a[:, m:m + 1], kb1, OP.mult)
            q = scp.tile([128, 1], F32, tag="al_q")
            nc.vector.tensor_tensor(q, p, p, OP.mult)
            z = scp.tile([128, 1], F32, tag="al_z")
            nc.vector.tensor_tensor(z, v, q, OP.mult)
            nc.scalar.sqrt(al1_a[:, m:m + 1], z)

        def mm1_side(m, kb1):
            alpha1_smalls(m, kb1)
            if has_b1:
                ia1 = scp.tile([128, 1], F32, tag="ia1")
                nc.vector.reciprocal(ia1, al1_a[:, m:m + 1])
                ia1_row = col_to_row(ia1, "ia1r")
            g_t = gp.tile([128, F], F32, tag="g")
            for fb in range(NF1 // FB):
                p1s = [ps1.tile([128, 512], F32, tag="ps1", name=f"p1_{fb}_{i}")
                       for i in range(FB)]
                for k in range(KD):
                    for fi in range(FB):
                        f = fb * FB + fi
                        nc.tensor.matmul(
                            p1s[fi], lhsT=n1T_aps[m][:, k, :],
                            rhs=w1qT[:, k, f * 512:(f + 1) * 512],
                            start=(k == 0), stop=(k == KD - 1 and not has_b1),
                        )
                for fi in range(FB):
                    f = fb * FB + fi
                    p1 = p1s[fi]
                    if has_b1:
                        bt1 = scp.tile([1, 512], F32, tag="bias", bufs=1)
                        nc.sync.dma_start(bt1, b1_d[:, f * 512:(f + 1) * 512])
                        nc.tensor.matmul(p1, lhsT=ia1_row, rhs=bt1,
                                         start=False, stop=True)
                    fs = slice(f * 512, (f + 1) * 512)
                    nc.scalar.activation(g_t[:, fs], p1, AF.Gelu,
                                         scale=al1_a[:, m:m + 1])
            # per-token absmax over g: one DVE reduce over the full row
            nc.vector.tensor_reduce(
                amax2_a[:, m:m + 1], g_t, axis=AX.X, op=OP.max,
                apply_absolute_value=True)
            rec2 = scp.tile([128, 1], F32, tag="rec2")
            nc.vector.reciprocal(rec2, amax2_a[:, m:m + 1])
            nc.vector.tensor_scalar_mul(c2_a[:, m:m + 1], rec2, 127.0)
            # round trick, optionally split across DVE and ACT halves
            FH = F // 2 if QSPLIT else 0
            n2 = n2p.tile([128, F], BF16, tag="n2")
            if FH:
                nc.vector.tensor_scalar(g_t[:, :FH], g_t[:, :FH],
                                        c2_a[:, m:m + 1], RND,
                                        op0=OP.mult, op1=OP.add)
                nc.vector.tensor_scalar(n2[:, :FH], g_t[:, :FH], -RND, None,
                                        op0=OP.add)
            nc.scalar.activation(g_t[:, FH:], g_t[:, FH:], AF.Copy, bias=RND,
                                 scale=c2_a[:, m:m + 1])
            nc.scalar.activation(n2[:, FH:], g_t[:, FH:], AF.Copy, bias=-RND)
            # n2T directly via SBUF->SBUF xbar transposes (no DRAM roundtrip);
            # 1024-wide chunks (same proven shape as the n1T transposes)
            n2T = n2tp.tile([128, KF, 128], BF16, tag="n2T")
            for q in range(F // 1024):
                nc.sync.dma_start_transpose(
                    n2T[:, q * 8:(q + 1) * 8, :],
                    n2[:, q * 1024:(q + 1) * 1024])
            n2T_aps[m] = n2T
            # sum(n2^2): square into ssink (bf16, DVE-serial so safe), reduce
            nc.vector.tensor_tensor(ssink, n2, n2, OP.mult)
            nc.vector.tensor_reduce(ssqn2_a[:, m:m + 1], ssink, axis=AX.X,
                                    op=OP.add)

        # ---- phase B ----
        n2T_aps = {}

        def alpha2_smalls(j, kb2):
            """alpha2 = kb2*amax2*rsqrt(mean(g^2)+eps);
            Sum(g^2) ~= Sum(n2^2)*(amax2/127)^2."""
            sq = ssqn2_a[:, j:j + 1]
            aa = scp.tile([128, 1], F32, tag="aa")
            nc.vector.tensor_tensor(aa, amax2_a[:, j:j + 1],
                                    amax2_a[:, j:j + 1], OP.mult)
            u = scp.tile([128, 1], F32, tag="al_u")
            nc.vector.tensor_tensor(u, sq, aa, OP.mult)
            nc.vector.tensor_scalar(u, u, 1.0 / (16129.0 * F), EPS,
                                    op0=OP.mult, op1=OP.add)
            v = scp.tile([128, 1], F32, tag="al_v")
            nc.vector.reciprocal(v, u)
            p = scp.tile([128, 1], F32, tag="al_p")
            nc.vector.tensor_tensor(p, amax2_a[:, j:j + 1], kb2, OP.mult)
            q = scp.tile([128, 1], F32, tag="al_q")
            nc.vector.tensor_tensor(q, p, p, OP.mult)
            z = scp.tile([128, 1], F32, tag="al_z")
            nc.vector.tensor_tensor(z, v, q, OP.mult)
            nc.scalar.sqrt(al2_a[:, j:j + 1], z)

        def mm2_side(j):
            n2T = n2T_aps.pop(j)
            if has_b2:
                ia2 = scp.tile([128, 1], F32, tag="ia2")
                nc.vector.reciprocal(ia2, al2_a[:, j:j + 1])
                ia2_row = col_to_row(ia2, "ia2r")
            o_t = outp.tile([128, D], F32, tag="o")
            p2s = [ps2.tile([128, 512], F32, tag="ps2", name=f"p2_{i}")
                   for i in range(ND2)]
            for k2 in range(KF):
                for d in range(ND2):
                    nc.tensor.matmul(
                        p2s[d], lhsT=n2T[:, k2, :],
                        rhs=w2qT[:, k2, d * 512:(d + 1) * 512],
                        start=(k2 == 0), stop=(k2 == KF - 1 and not has_b2),
                    )
            for d in range(ND2):
                p2 = p2s[d]
                if has_b2:
                    bt2 = scp.tile([1, 512], F32, tag="bias", bufs=1)
                    nc.sync.dma_start(bt2, b2_d[:, d * 512:(d + 1) * 512])
                    nc.tensor.matmul(p2, lhsT=ia2_row, rhs=bt2,
                                     start=False, stop=True)
                ds_ = slice(d * 512, (d + 1) * 512)
                nc.scalar.activation(o_t[:, ds_], p2, AF.Copy,
                                     scale=al2_a[:, j:j + 1])
            nc.sync.dma_start(out_d[j * 128:(j + 1) * 128, :], o_t)

        # ================= emission =================
        rep_ctx = tc.For_i(0, REPEAT, 1) if REPEAT > 1 else None
        if rep_ctx is not None:
            ctx.enter_context(rep_ctx)
        x_prep_chunk(0)
        # w1 pass 1
        for c in range(NW1C):
            w_abs_chunk(w1_src(c), w1part[:, c:c + 1])
        s1b, kb1 = finalize_mean(w1part, F * D, "w1")
        if NXC > 1:
            x_prep_chunk(1)
        # w1 pass 2 interleaved with w2 pass 1 (w2qT must be complete by
        # phase-B start at it=LAG, so pass 1 belongs in the prologue)
        for c in range(max(NW1C, NW2C)):
            if c < NW1C:
                tern_chunk(w1_src(c), s1b, w1_dsts(c))
            if c < NW2C:
                w_abs_chunk(w2_src(c), w2part[:, c:c + 1])
        s2b, kb2 = finalize_mean(w2part, D * F, "w2")

        # phase A with interleaved w2 pass 2, phase B interleaved at LAG
        LAG = min(LAG_P, TT)
        w2_per = max(1, (NW2C + LAG - 2) // (LAG - 1)) if LAG > 1 else NW2C
        w2p2_next = 0
        for it in range(TT + LAG):
            if it < TT:
                mc = it // XCH + 2
                if it % XCH == 0 and mc < NXC:
                    x_prep_chunk(mc)
                for _ in range(w2_per):
                    if w2p2_next < NW2C:
                        tern_chunk(w2_src(w2p2_next), s2b,
                                   w2_dsts(w2p2_next),
                                   cast_eng=nc.gpsimd.tensor_copy,
                                   ts_eng=(nc.gpsimd if w2p2_next % 2 else None))
                        w2p2_next += 1
                mm1_side(it, kb1)
            j = it - LAG
            if 0 <= j < TT:
                alpha2_smalls(j, kb2)
                mm2_side(j)

    nc.compile()
    return nc


_NC_CACHE = {}


def _get_nc(T_core, D, F, has_b1, has_b2):
    key = (T_core, D, F, has_b1, has_b2)
    if key not in _NC_CACHE:
        _NC_CACHE[key] = build_bitffn(T_core, D, F, has_b1, has_b2)
    return _NC_CACHE[key]


def kernel(x, w1, b1, w2, b2):
    B, S, D = x.shape
    Fdim = w1.shape[0]
    T = B * S
    T_core = T // NCORES
    has_b1 = bool(np.any(b1))
    has_b2 = bool(np.any(b2))

    nc = _get_nc(T_core, D, Fdim, has_b1, has_b2)

    xf = np.ascontiguousarray(x.reshape(T, D).astype(np.float32))
    w1c = np.ascontiguousarray(w1.astype(np.float32))
    w2c = np.ascontiguousarray(w2.astype(np.float32))
    b1c = np.ascontiguousarray(b1.reshape(1, Fdim).astype(np.float32))
    b2c = np.ascontiguousarray(b2.reshape(1, D).astype(np.float32))

    in_maps = [
        {
            "x": xf[i * T_core:(i + 1) * T_core],
            "w1": w1c,
            "b1": b1c,
            "w2": w2c,
            "b2": b2c,
        }
        for i in range(NCORES)
    ]
    res = bass_utils.run_bass_kernel_spmd(
        nc, in_maps=in_maps, core_ids=list(range(NCORES))
    )
    out = np.concatenate([res.results[i]["out"] for i in range(NCORES)], axis=0)
    return out.reshape(B, S, D).astype(np.float32)
